# revision 3
# baseline (speedup 1.0000x reference)
"""Trainium2 Bass kernel for nn_ContrasTR memory compaction/scatter.

Operation (per batch element, fully independent):
  - evict stale memory rows (frame_number <= current-5 or ~mask), stream-compact
    survivors to the front
  - append predictions whose max class sigmoid >= 0.8 (max logit >= logit(0.8))
  - zero/invalidate the tail; emit mask and nb_items

Device algorithm (data parallel, 8 batch elements per NeuronCore x 8 cores):
  - host packs "wide rows" (emb[256] | fn_bits | tid_bits | fn_f32 | mask_f32)
    = 260 f32 for 5760 rows (4500 mem + 900 pred + 360 pad) per element,
    p-major layout: row i -> SBUF partition i//45, column i%45
  - on device: keep-bits via DVE compares (pred bits via class-max reduce),
    inclusive prefix sum (free-dim scan + exact cross-partition scan using
    PE transposes, all sums on DVE so integer-exact), then one indirect
    scatter-DMA of surviving rows (OOB bounds-check silently drops evicted
    rows and capacity overflow), plus a second scatter writing constant
    (0...,-1,0) tail rows for j >= nb_items
  - mask/nb_items computed directly from the totals
"""
import math
import sys
from contextlib import ExitStack

import numpy as np

try:
    import concourse.bass as bass
except ImportError:  # grading env may not have it on sys.path yet
    sys.path.insert(0, "/opt/trn_rl_repo")
    import concourse.bass as bass

import concourse.bacc as bacc
import concourse.mybir as mybir
import concourse.tile as tile
from concourse.bass_utils import run_bass_kernel_spmd
from concourse.masks import make_identity

N_CORES = 8
B = 64
NBE = B // N_CORES  # batch elements per core

P = 128
S = 45              # free-dim width of bit layout; P*S = 5760 rows
R = P * S           # 5760
M = 4500            # memory capacity per batch element
Q = 900
C = 91
D = 256
CPP = 45 * C        # classes per partition for the reduce = 4095
RW = 260            # wide row in f32
THR = float(np.log(np.float32(0.8) / np.float32(0.2)))  # logit(0.8)
BIG = 100000.0      # skip sentinel for dropped rows

f32 = mybir.dt.float32
i32 = mybir.dt.int32
u8 = mybir.dt.uint8
Alu = mybir.AluOpType


def build(nbe: int = NBE) -> bass.Bass:
    nc = bacc.Bacc("TRN2")

    srcw = nc.dram_tensor("srcw", [nbe * R, RW], f32, kind="ExternalInput")
    cls = nc.dram_tensor("cls", [nbe, 20, CPP], f32, kind="ExternalInput")
    thr = nc.dram_tensor("thr", [nbe, P, 1], f32, kind="ExternalInput")
    outw = nc.dram_tensor("outw", [nbe * M, RW], f32, kind="ExternalOutput")
    outm = nc.dram_tensor("outm", [nbe, M], u8, kind="ExternalOutput")
    outn = nc.dram_tensor("outn", [nbe, 1], i32, kind="ExternalOutput")

    srcw_r = srcw[:].rearrange("(b p c) w -> b p (c w)", b=nbe, p=P)

    with tile.TileContext(nc) as tc, ExitStack() as ctx:
        const = ctx.enter_context(tc.tile_pool(name="const", bufs=1))
        big = ctx.enter_context(tc.tile_pool(name="big", bufs=2))
        sm = ctx.enter_context(tc.tile_pool(name="sm", bufs=3))
        ps = ctx.enter_context(tc.tile_pool(name="ps", bufs=2, space="PSUM"))

        # ---- one-time constants ----
        ident = const.tile([P, P], f32)
        make_identity(nc, ident[:])

        # E127[k, p] = 1.0 iff k == 127 (exact single-product broadcast matmul)
        rowi = const.tile([P, P], i32)
        nc.gpsimd.iota(rowi[:], pattern=[[0, P]], base=0, channel_multiplier=1)
        e127i = const.tile([P, P], i32)
        nc.vector.tensor_scalar(
            out=e127i[:], in0=rowi[:], scalar1=127, scalar2=None, op0=Alu.is_equal
        )
        e127 = const.tile([P, P], f32)
        nc.vector.tensor_copy(e127[:], e127i[:])

        # j = p*S + c as f32
        jti = const.tile([P, S], i32)
        nc.gpsimd.iota(jti[:], pattern=[[1, S]], base=0, channel_multiplier=S)
        jtf = const.tile([P, S], f32)
        nc.vector.tensor_copy(jtf[:], jti[:])


        for be in range(nbe):
            w = big.tile([P, S * RW], f32, tag="w")
            nc.sync.dma_start(out=w[:], in_=srcw_r[be])
            wv = w[:].rearrange("p (c w) -> p c w", w=RW)

            cc = big.tile([P, CPP], f32, tag="cc")
            nc.sync.dma_start(out=cc[:20, :], in_=cls[be])

            th = sm.tile([P, 1], f32, tag="th")
            nc.sync.dma_start(out=th[:], in_=thr[be])

            # mem keep bits: (fn > cur-5) * mask  (pred/pad rows have 0 fields)
            bits = sm.tile([P, S], f32, tag="bits")
            nc.vector.scalar_tensor_tensor(
                out=bits[:], in0=wv[:, :, 258], scalar=th[:, :1], in1=wv[:, :, 259],
                op0=Alu.is_gt, op1=Alu.mult,
            )
            # pred bits: reduce/threshold at partitions 0..19 (DVE partition
            # base must be %32), then DMA into layout partitions 100..119
            mx = sm.tile([P, S], f32, tag="mx")
            nc.vector.tensor_reduce(
                out=mx[:20, :],
                in_=cc[:20, :].rearrange("p (c k) -> p c k", k=C),
                axis=mybir.AxisListType.X, op=Alu.max,
            )
            bitsp = sm.tile([P, S], f32, tag="bitsp")
            nc.vector.tensor_scalar(
                out=bitsp[:20, :], in0=mx[:20, :], scalar1=THR, scalar2=None,
                op0=Alu.is_ge,
            )
            nc.sync.dma_start(out=bits[100:120, :], in_=bitsp[:20, :])

            # within-partition inclusive prefix
            sc = sm.tile([P, S], f32, tag="sc")
            nc.vector.tensor_tensor_scan(
                out=sc[:], data0=bits[:], data1=bits[:], initial=0.0,
                op0=Alu.add, op1=Alu.bypass,
            )

            # cross-partition inclusive prefix of per-partition totals (exact):
            # transpose totals to one partition, scan, transpose back.
            segt_ps = ps.tile([1, P], f32, tag="segt")
            nc.tensor.transpose(out=segt_ps[:], in_=sc[:, S - 1 : S], identity=ident[:])
            segt = sm.tile([1, P], f32, tag="segs")
            nc.vector.tensor_copy(segt[:], segt_ps[:])
            pinc_t = sm.tile([1, P], f32, tag="pinct")
            nc.vector.tensor_tensor_scan(
                out=pinc_t[:], data0=segt[:], data1=segt[:], initial=0.0,
                op0=Alu.add, op1=Alu.bypass,
            )
            pinc_ps = ps.tile([P, 1], f32, tag="pinc")
            nc.tensor.transpose(out=pinc_ps[:], in_=pinc_t[:], identity=ident[:1, :1])
            pinc = sm.tile([P, 1], f32, tag="pincs")
            nc.vector.tensor_copy(pinc[:], pinc_ps[:])

            # exclusive offset (+ -1 for 0-based dest): off = pinc - seg_tot - 1
            off = sm.tile([P, 1], f32, tag="off")
            nc.vector.scalar_tensor_tensor(
                out=off[:], in0=pinc[:], scalar=-1.0, in1=sc[:, S - 1 : S],
                op0=Alu.add, op1=Alu.subtract,
            )

            # dest = sc + off where bit set, else BIG
            dpos = sm.tile([P, S], f32, tag="dpos")
            nc.vector.scalar_tensor_tensor(
                out=dpos[:], in0=sc[:], scalar=off[:, :1], in1=sc[:],
                op0=Alu.add, op1=Alu.bypass,
            )
            bitsi = sm.tile([P, S], i32, tag="bitsi")
            nc.vector.tensor_copy(bitsi[:], bits[:])
            idxf = sm.tile([P, S], f32, tag="idxf")
            nc.vector.memset(idxf[:], BIG)
            nc.vector.copy_predicated(out=idxf[:], mask=bitsi[:], data=dpos[:])
            idxi = sm.tile([P, S], i32, tag="idxi")
            nc.vector.tensor_copy(idxi[:], idxf[:])

            # main scatter: surviving rows -> compacted destinations.
            # HW SWDGE supports one index per partition, so issue one
            # indirect DMA per column (128 rows each); dropped rows and
            # capacity overflow are skipped via the OOB bounds check.
            # Tail rows stay at the output buffer's pre-zeroed state.
            for c in range(S):
                nc.gpsimd.indirect_dma_start(
                    out=outw[:],
                    out_offset=bass.IndirectOffsetOnAxis(ap=idxi[:, c : c + 1], axis=0),
                    in_=wv[:, c, :],
                    in_offset=None,
                    element_offset=be * M * RW,
                    bounds_check=M - 1,
                    oob_is_err=False,
                )

            # nb_items = min(total, M), broadcast to all partitions (exact matmul)
            totm = sm.tile([P, 1], f32, tag="totm")
            nc.vector.tensor_scalar(
                out=totm[:], in0=pinc[:], scalar1=float(M), scalar2=None, op0=Alu.min
            )
            totb_ps = ps.tile([P, 1], f32, tag="totb")
            nc.tensor.matmul(totb_ps[:], lhsT=e127[:], rhs=totm[:], start=True, stop=True)
            totb = sm.tile([P, 1], f32, tag="totbs")
            nc.vector.tensor_copy(totb[:], totb_ps[:])


            # mask output: j < nb_items (only partitions 0..99 hold j < 4500)
            mkf = sm.tile([P, S], f32, tag="mkf")
            nc.vector.scalar_tensor_tensor(
                out=mkf[:], in0=jtf[:], scalar=totb[:, :1], in1=jtf[:],
                op0=Alu.is_lt, op1=Alu.bypass,
            )
            mk8 = sm.tile([P, S], u8, tag="mk8")
            nc.vector.tensor_copy(mk8[:], mkf[:])
            nc.sync.dma_start(
                out=outm[be].rearrange("(p c) -> p c", c=S), in_=mk8[:100, :]
            )

            nbi = sm.tile([1, 1], i32, tag="nbi")
            nc.vector.tensor_copy(nbi[:], totb[:1, :1])
            nc.sync.dma_start(out=outn[be], in_=nbi[:])

    nc.compile()
    return nc


_NC_CACHE = {}


def _get_nc():
    if "nc" not in _NC_CACHE:
        _NC_CACHE["nc"] = build(NBE)
    return _NC_CACHE["nc"]


def _pack_core(mem_emb, fn, tid, msk, p_emb, p_cls, p_id, cur):
    """Pack one core's shard (NBE elements) into kernel input arrays."""
    nbe = mem_emb.shape[0]
    srcw = np.zeros((nbe, R, RW), np.float32)
    srcw[:, :M, :D] = mem_emb
    srcw[:, :M, 256] = np.ascontiguousarray(fn).view(np.float32)
    srcw[:, :M, 257] = np.ascontiguousarray(tid).view(np.float32)
    srcw[:, :M, 258] = fn.astype(np.float32)
    srcw[:, :M, 259] = msk.astype(np.float32)
    srcw[:, M : M + Q, :D] = p_emb
    curb = np.ascontiguousarray(np.broadcast_to(cur[:, None], (nbe, Q)).astype(np.int32))
    srcw[:, M : M + Q, 256] = curb.view(np.float32)
    srcw[:, M : M + Q, 257] = np.ascontiguousarray(p_id).view(np.float32)
    clsx = np.ascontiguousarray(p_cls.reshape(nbe, 20, CPP))
    thrx = np.ascontiguousarray(
        np.broadcast_to((cur.astype(np.float32) - 5.0)[:, None, None], (nbe, P, 1))
    )
    return {"srcw": srcw.reshape(nbe * R, RW), "cls": clsx, "thr": thrx}


def _unpack_core(res, nbe=NBE):
    ow = res["outw"].reshape(nbe, M, RW)
    emb = np.ascontiguousarray(ow[:, :, :D])
    fn = np.ascontiguousarray(ow[:, :, 256]).view(np.int32)
    tid = np.ascontiguousarray(ow[:, :, 257]).view(np.int32)
    mask = res["outm"].astype(bool)
    nb = res["outn"].reshape(nbe).astype(np.int32)
    fn = np.where(mask, fn, -1)  # tail rows keep the buffer's zero fill; fn tail is -1
    return emb, fn, tid, mask, nb


def run(inputs, trace=False, trace_kwargs=None):
    """Shard across 8 cores, run the bass kernel, gather. Returns
    ((emb, fn, tid, mask, nb_items), BassKernelResults)."""
    nc = _get_nc()
    in_maps = []
    for c in range(N_CORES):
        sl = slice(c * NBE, (c + 1) * NBE)
        in_maps.append(
            _pack_core(
                inputs["mem_embeddings"][sl],
                inputs["mem_frame_numbers"][sl],
                inputs["mem_tracking_ids"][sl],
                inputs["mem_mask"][sl],
                inputs["pred_embedds"][sl],
                inputs["pred_classes"][sl],
                inputs["pred_ids"][sl],
                inputs["current_frames"][sl],
            )
        )
    br = run_bass_kernel_spmd(
        nc, in_maps, core_ids=list(range(N_CORES)), trace=trace,
        **({"trace_kwargs": trace_kwargs} if trace_kwargs else {}),
    )
    parts = [_unpack_core(r) for r in br.results]
    emb = np.concatenate([p[0] for p in parts], axis=0)
    fn = np.concatenate([p[1] for p in parts], axis=0)
    tid = np.concatenate([p[2] for p in parts], axis=0)
    mask = np.concatenate([p[3] for p in parts], axis=0)
    nb = np.concatenate([p[4] for p in parts], axis=0)
    return (emb, fn, tid, mask, nb), br


def kernel(**inputs):
    outs, _ = run(inputs, trace=False)
    return outs


# revision 4
# speedup vs baseline: 1.6100x; 1.6100x over previous
"""Trainium2 Bass kernel for nn_ContrasTR memory compaction/scatter.

Operation (per batch element, fully independent):
  - evict stale memory rows (frame_number <= current-5 or ~mask), stream-compact
    survivors to the front
  - append predictions whose max class sigmoid >= 0.8 (max logit >= logit(0.8))
  - zero/invalidate the tail; emit mask and nb_items

Device algorithm (data parallel, 8 batch elements per NeuronCore x 8 cores):
  - host packs "wide rows" (emb[256] | fn_bits | tid_bits | fn_f32 | mask_f32)
    = 260 f32 for 5760 rows (4500 mem + 900 pred + 360 pad) per element,
    p-major layout: row i -> SBUF partition i//45, column i%45
  - on device: keep-bits via DVE compares (pred bits via class-max reduce),
    inclusive prefix sum (free-dim scan + exact cross-partition scan using
    PE transposes, all sums on DVE so integer-exact), then one indirect
    scatter-DMA of surviving rows (OOB bounds-check silently drops evicted
    rows and capacity overflow), plus a second scatter writing constant
    (0...,-1,0) tail rows for j >= nb_items
  - mask/nb_items computed directly from the totals
"""
import math
import sys
from contextlib import ExitStack

import numpy as np

try:
    import concourse.bass as bass
except ImportError:  # grading env may not have it on sys.path yet
    sys.path.insert(0, "/opt/trn_rl_repo")
    import concourse.bass as bass

import concourse.bacc as bacc
import concourse.mybir as mybir
import concourse.tile as tile
from concourse.bass_utils import run_bass_kernel_spmd
from concourse.masks import make_identity

N_CORES = 8
B = 64
NBE = B // N_CORES  # batch elements per core

P = 128
S = 45              # free-dim width of bit layout; P*S = 5760 rows
R = P * S           # 5760
M = 4500            # memory capacity per batch element
Q = 900
C = 91
D = 256
CPP = 45 * C        # classes per partition for the reduce = 4095
RW = 260            # wide row in f32
THR = float(np.log(np.float32(0.8) / np.float32(0.2)))  # logit(0.8)
BIG = 100000.0      # skip sentinel for dropped rows

f32 = mybir.dt.float32
i32 = mybir.dt.int32
u8 = mybir.dt.uint8
Alu = mybir.AluOpType


def build(nbe: int = NBE) -> bass.Bass:
    nc = bacc.Bacc("TRN2")

    srcw = nc.dram_tensor("srcw", [nbe * R, RW], f32, kind="ExternalInput")
    cls = nc.dram_tensor("cls", [nbe, 20, CPP], f32, kind="ExternalInput")
    thr = nc.dram_tensor("thr", [nbe, P, 1], f32, kind="ExternalInput")
    outw_t = [
        nc.dram_tensor(f"outw{b}", [M, RW], f32, kind="ExternalOutput")
        for b in range(nbe)
    ]
    outm = nc.dram_tensor("outm", [nbe, M], u8, kind="ExternalOutput")
    outn = nc.dram_tensor("outn", [nbe, 1], i32, kind="ExternalOutput")

    srcw_r = srcw[:].rearrange("(b p c) w -> b p (c w)", b=nbe, p=P)

    with tile.TileContext(nc) as tc, ExitStack() as ctx:
        const = ctx.enter_context(tc.tile_pool(name="const", bufs=1))
        big = ctx.enter_context(tc.tile_pool(name="big", bufs=2))
        sm = ctx.enter_context(tc.tile_pool(name="sm", bufs=3))
        ps = ctx.enter_context(tc.tile_pool(name="ps", bufs=2, space="PSUM"))

        # ---- one-time constants ----
        ident = const.tile([P, P], f32)
        make_identity(nc, ident[:])

        # E127[k, p] = 1.0 iff k == 127 (exact single-product broadcast matmul)
        rowi = const.tile([P, P], i32)
        nc.gpsimd.iota(rowi[:], pattern=[[0, P]], base=0, channel_multiplier=1)
        e127i = const.tile([P, P], i32)
        nc.vector.tensor_scalar(
            out=e127i[:], in0=rowi[:], scalar1=127, scalar2=None, op0=Alu.is_equal
        )
        e127 = const.tile([P, P], f32)
        nc.vector.tensor_copy(e127[:], e127i[:])

        # j = p*S + c as f32
        jti = const.tile([P, S], i32)
        nc.gpsimd.iota(jti[:], pattern=[[1, S]], base=0, channel_multiplier=S)
        jtf = const.tile([P, S], f32)
        nc.vector.tensor_copy(jtf[:], jti[:])


        for be in range(nbe):
            w = big.tile([P, S * RW], f32, tag="w")
            nc.sync.dma_start(out=w[:], in_=srcw_r[be])
            wv = w[:].rearrange("p (c w) -> p c w", w=RW)

            cc = big.tile([P, CPP], f32, tag="cc")
            nc.sync.dma_start(out=cc[:20, :], in_=cls[be])

            th = sm.tile([P, 1], f32, tag="th")
            nc.sync.dma_start(out=th[:], in_=thr[be])

            # mem keep bits: (fn > cur-5) * mask  (pred/pad rows have 0 fields)
            bits = sm.tile([P, S], f32, tag="bits")
            nc.vector.scalar_tensor_tensor(
                out=bits[:], in0=wv[:, :, 258], scalar=th[:, :1], in1=wv[:, :, 259],
                op0=Alu.is_gt, op1=Alu.mult,
            )
            # pred bits: reduce/threshold at partitions 0..19 (DVE partition
            # base must be %32), then DMA into layout partitions 100..119
            mx = sm.tile([P, S], f32, tag="mx")
            nc.vector.tensor_reduce(
                out=mx[:20, :],
                in_=cc[:20, :].rearrange("p (c k) -> p c k", k=C),
                axis=mybir.AxisListType.X, op=Alu.max,
            )
            bitsp = sm.tile([P, S], f32, tag="bitsp")
            nc.vector.tensor_scalar(
                out=bitsp[:20, :], in0=mx[:20, :], scalar1=THR, scalar2=None,
                op0=Alu.is_ge,
            )
            nc.sync.dma_start(out=bits[100:120, :], in_=bitsp[:20, :])

            # within-partition inclusive prefix
            sc = sm.tile([P, S], f32, tag="sc")
            nc.vector.tensor_tensor_scan(
                out=sc[:], data0=bits[:], data1=bits[:], initial=0.0,
                op0=Alu.add, op1=Alu.bypass,
            )

            # cross-partition inclusive prefix of per-partition totals (exact):
            # transpose totals to one partition, scan, transpose back.
            segt_ps = ps.tile([1, P], f32, tag="segt")
            nc.tensor.transpose(out=segt_ps[:], in_=sc[:, S - 1 : S], identity=ident[:])
            segt = sm.tile([1, P], f32, tag="segs")
            nc.vector.tensor_copy(segt[:], segt_ps[:])
            pinc_t = sm.tile([1, P], f32, tag="pinct")
            nc.vector.tensor_tensor_scan(
                out=pinc_t[:], data0=segt[:], data1=segt[:], initial=0.0,
                op0=Alu.add, op1=Alu.bypass,
            )
            pinc_ps = ps.tile([P, 1], f32, tag="pinc")
            nc.tensor.transpose(out=pinc_ps[:], in_=pinc_t[:], identity=ident[:1, :1])
            pinc = sm.tile([P, 1], f32, tag="pincs")
            nc.vector.tensor_copy(pinc[:], pinc_ps[:])

            # exclusive offset (+ -1 for 0-based dest): off = pinc - seg_tot - 1
            off = sm.tile([P, 1], f32, tag="off")
            nc.vector.scalar_tensor_tensor(
                out=off[:], in0=pinc[:], scalar=-1.0, in1=sc[:, S - 1 : S],
                op0=Alu.add, op1=Alu.subtract,
            )

            # dest = sc + off where bit set, else BIG
            dpos = sm.tile([P, S], f32, tag="dpos")
            nc.vector.scalar_tensor_tensor(
                out=dpos[:], in0=sc[:], scalar=off[:, :1], in1=sc[:],
                op0=Alu.add, op1=Alu.bypass,
            )
            bitsi = sm.tile([P, S], i32, tag="bitsi")
            nc.vector.tensor_copy(bitsi[:], bits[:])
            idxf = sm.tile([P, S], f32, tag="idxf")
            nc.vector.memset(idxf[:], BIG)
            nc.vector.copy_predicated(out=idxf[:], mask=bitsi[:], data=dpos[:])
            idxi = sm.tile([P, S], i32, tag="idxi")
            nc.vector.tensor_copy(idxi[:], idxf[:])

            # main scatter: surviving rows -> compacted destinations.
            # HW SWDGE supports one index per partition, so issue one
            # indirect DMA per column (128 rows each); dropped rows and
            # capacity overflow are skipped via the OOB bounds check.
            # Tail rows stay at the output buffer's pre-zeroed state.
            for c in range(S):
                nc.gpsimd.indirect_dma_start(
                    out=outw_t[be][:],
                    out_offset=bass.IndirectOffsetOnAxis(ap=idxi[:, c : c + 1], axis=0),
                    in_=wv[:, c, :],
                    in_offset=None,
                    bounds_check=M - 1,
                    oob_is_err=False,
                )

            # nb_items = min(total, M), broadcast to all partitions (exact matmul)
            totm = sm.tile([P, 1], f32, tag="totm")
            nc.vector.tensor_scalar(
                out=totm[:], in0=pinc[:], scalar1=float(M), scalar2=None, op0=Alu.min
            )
            totb_ps = ps.tile([P, 1], f32, tag="totb")
            nc.tensor.matmul(totb_ps[:], lhsT=e127[:], rhs=totm[:], start=True, stop=True)
            totb = sm.tile([P, 1], f32, tag="totbs")
            nc.vector.tensor_copy(totb[:], totb_ps[:])


            # mask output: j < nb_items (only partitions 0..99 hold j < 4500)
            mkf = sm.tile([P, S], f32, tag="mkf")
            nc.vector.scalar_tensor_tensor(
                out=mkf[:], in0=jtf[:], scalar=totb[:, :1], in1=jtf[:],
                op0=Alu.is_lt, op1=Alu.bypass,
            )
            mk8 = sm.tile([P, S], u8, tag="mk8")
            nc.vector.tensor_copy(mk8[:], mkf[:])
            nc.sync.dma_start(
                out=outm[be].rearrange("(p c) -> p c", c=S), in_=mk8[:100, :]
            )

            nbi = sm.tile([1, 1], i32, tag="nbi")
            nc.vector.tensor_copy(nbi[:], totb[:1, :1])
            nc.sync.dma_start(out=outn[be], in_=nbi[:])

    nc.compile()
    return nc


_NC_CACHE = {}


def _get_nc():
    if "nc" not in _NC_CACHE:
        _NC_CACHE["nc"] = build(NBE)
    return _NC_CACHE["nc"]


def _pack_core(mem_emb, fn, tid, msk, p_emb, p_cls, p_id, cur):
    """Pack one core's shard (NBE elements) into kernel input arrays."""
    nbe = mem_emb.shape[0]
    srcw = np.zeros((nbe, R, RW), np.float32)
    srcw[:, :M, :D] = mem_emb
    srcw[:, :M, 256] = np.ascontiguousarray(fn).view(np.float32)
    srcw[:, :M, 257] = np.ascontiguousarray(tid).view(np.float32)
    srcw[:, :M, 258] = fn.astype(np.float32)
    srcw[:, :M, 259] = msk.astype(np.float32)
    srcw[:, M : M + Q, :D] = p_emb
    curb = np.ascontiguousarray(np.broadcast_to(cur[:, None], (nbe, Q)).astype(np.int32))
    srcw[:, M : M + Q, 256] = curb.view(np.float32)
    srcw[:, M : M + Q, 257] = np.ascontiguousarray(p_id).view(np.float32)
    clsx = np.ascontiguousarray(p_cls.reshape(nbe, 20, CPP))
    thrx = np.ascontiguousarray(
        np.broadcast_to((cur.astype(np.float32) - 5.0)[:, None, None], (nbe, P, 1))
    )
    return {"srcw": srcw.reshape(nbe * R, RW), "cls": clsx, "thr": thrx}


def _unpack_core(res, nbe=NBE):
    ow = np.stack([res[f"outw{b}"] for b in range(nbe)], axis=0)
    emb = np.ascontiguousarray(ow[:, :, :D])
    fn = np.ascontiguousarray(ow[:, :, 256]).view(np.int32)
    tid = np.ascontiguousarray(ow[:, :, 257]).view(np.int32)
    mask = res["outm"].astype(bool)
    nb = res["outn"].reshape(nbe).astype(np.int32)
    fn = np.where(mask, fn, -1)  # tail rows keep the buffer's zero fill; fn tail is -1
    return emb, fn, tid, mask, nb


def run(inputs, trace=False, trace_kwargs=None):
    """Shard across 8 cores, run the bass kernel, gather. Returns
    ((emb, fn, tid, mask, nb_items), BassKernelResults)."""
    nc = _get_nc()
    in_maps = []
    for c in range(N_CORES):
        sl = slice(c * NBE, (c + 1) * NBE)
        in_maps.append(
            _pack_core(
                inputs["mem_embeddings"][sl],
                inputs["mem_frame_numbers"][sl],
                inputs["mem_tracking_ids"][sl],
                inputs["mem_mask"][sl],
                inputs["pred_embedds"][sl],
                inputs["pred_classes"][sl],
                inputs["pred_ids"][sl],
                inputs["current_frames"][sl],
            )
        )
    br = run_bass_kernel_spmd(
        nc, in_maps, core_ids=list(range(N_CORES)), trace=trace,
        **({"trace_kwargs": trace_kwargs} if trace_kwargs else {}),
    )
    parts = [_unpack_core(r) for r in br.results]
    emb = np.concatenate([p[0] for p in parts], axis=0)
    fn = np.concatenate([p[1] for p in parts], axis=0)
    tid = np.concatenate([p[2] for p in parts], axis=0)
    mask = np.concatenate([p[3] for p in parts], axis=0)
    nb = np.concatenate([p[4] for p in parts], axis=0)
    return (emb, fn, tid, mask, nb), br


def kernel(**inputs):
    outs, _ = run(inputs, trace=False)
    return outs


# revision 8
# speedup vs baseline: 1.7242x; 1.0709x over previous
"""Trainium2 Bass kernel for nn_ContrasTR memory compaction/scatter.

Operation (per batch element, fully independent):
  - evict stale memory rows (frame_number <= current-5 or ~mask), stream-compact
    survivors to the front
  - append predictions whose max class sigmoid >= 0.8 (max logit >= logit(0.8))
  - zero/invalidate the tail; emit mask and nb_items

Device algorithm (data parallel, 8 batch elements per NeuronCore x 8 cores):
  - host packs "wide rows" (emb[256] | fn_bits | tid_bits | fn_f32 | mask_f32)
    = 260 f32 for 5760 rows (4500 mem + 900 pred + 360 pad) per element,
    p-major layout: row i -> SBUF partition i//45, column i%45
  - on device: keep-bits via DVE compares (pred bits via class-max reduce),
    inclusive prefix sum (free-dim scan + exact cross-partition scan using
    PE transposes, all sums on DVE so integer-exact), then one indirect
    scatter-DMA of surviving rows (OOB bounds-check silently drops evicted
    rows and capacity overflow), plus a second scatter writing constant
    (0...,-1,0) tail rows for j >= nb_items
  - mask/nb_items computed directly from the totals
"""
import math
import sys
from contextlib import ExitStack

import numpy as np

try:
    import concourse.bass as bass
except ImportError:  # grading env may not have it on sys.path yet
    sys.path.insert(0, "/opt/trn_rl_repo")
    import concourse.bass as bass

import concourse.bacc as bacc
import concourse.mybir as mybir
import concourse.tile as tile
from concourse.bass_utils import run_bass_kernel_spmd
from concourse.masks import make_identity

N_CORES = 8
B = 64
NBE = B // N_CORES  # batch elements per core

P = 128
S = 45              # free-dim width of bit layout; P*S = 5760 rows
R = P * S           # 5760
M = 4500            # memory capacity per batch element
Q = 900
C = 91
D = 256
CPP = 45 * C        # classes per partition for the reduce = 4095
RW = 260            # wide row in f32
THR = float(np.log(np.float32(0.8) / np.float32(0.2)))  # logit(0.8)
BIG = 100000.0      # skip sentinel for dropped rows

f32 = mybir.dt.float32
i32 = mybir.dt.int32
u8 = mybir.dt.uint8
Alu = mybir.AluOpType


def build(nbe: int = NBE) -> bass.Bass:
    nc = bacc.Bacc("TRN2")

    srcw = nc.dram_tensor("srcw", [nbe * R, RW], f32, kind="ExternalInput")
    cls = nc.dram_tensor("cls", [nbe, 100, 9 * C], f32, kind="ExternalInput")
    thr = nc.dram_tensor("thr", [nbe, P, 1], f32, kind="ExternalInput")
    outw_t = [
        nc.dram_tensor(f"outw{b}", [M, RW], f32, kind="ExternalOutput")
        for b in range(nbe)
    ]
    outm = nc.dram_tensor("outm", [nbe, M], u8, kind="ExternalOutput")
    outn = nc.dram_tensor("outn", [nbe, 1], i32, kind="ExternalOutput")

    srcw_r = srcw[:].rearrange("(b p c) w -> b p (c w)", b=nbe, p=P)

    with tile.TileContext(nc) as tc, ExitStack() as ctx:
        const = ctx.enter_context(tc.tile_pool(name="const", bufs=1))
        wp = ctx.enter_context(tc.tile_pool(name="wp", bufs=4))
        ccp = ctx.enter_context(tc.tile_pool(name="ccp", bufs=1))
        sm = ctx.enter_context(tc.tile_pool(name="sm", bufs=2))
        ps = ctx.enter_context(tc.tile_pool(name="ps", bufs=2, space="PSUM"))

        # ---- one-time constants ----
        ident = const.tile([P, P], f32)
        make_identity(nc, ident[:])

        # E127[k, p] = 1.0 iff k == 127 (exact single-product broadcast matmul)
        rowi = const.tile([P, P], i32)
        nc.gpsimd.iota(rowi[:], pattern=[[0, P]], base=0, channel_multiplier=1)
        e127i = const.tile([P, P], i32)
        nc.vector.tensor_scalar(
            out=e127i[:], in0=rowi[:], scalar1=127, scalar2=None, op0=Alu.is_equal
        )
        e127 = const.tile([P, P], f32)
        nc.vector.tensor_copy(e127[:], e127i[:])

        # one shared bounds-check register for all scatter calls
        breg = ctx.enter_context(nc.gpsimd.register("bnd"))
        nc.gpsimd.reg_mov(breg, M - 1)

        # j = p*S + c as f32
        jti = const.tile([P, S], i32)
        nc.gpsimd.iota(jti[:], pattern=[[1, S]], base=0, channel_multiplier=S)
        jtf = const.tile([P, S], f32)
        nc.vector.tensor_copy(jtf[:], jti[:])

        def element_front(be):
            """Load + keep-bits + prefix sums + scatter indices for one element."""
            w = wp.tile([P, S * RW], f32, tag="w")
            nc.sync.dma_start(out=w[:], in_=srcw_r[be])
            wv = w[:].rearrange("p (c w) -> p c w", w=RW)

            cc = ccp.tile([P, 9 * C], f32, tag="cc")
            nc.sync.dma_start(out=cc[:100, :], in_=cls[be])

            th = sm.tile([P, 1], f32, tag="th")
            nc.sync.dma_start(out=th[:], in_=thr[be])

            # mem keep bits: (fn > cur-5) * mask  (pred/pad rows have 0 fields)
            bits = sm.tile([P, S], f32, tag="bits")
            nc.vector.scalar_tensor_tensor(
                out=bits[:], in0=wv[:, :, 258], scalar=th[:, :1], in1=wv[:, :, 259],
                op0=Alu.is_gt, op1=Alu.mult,
            )
            # pred bits: reduce/threshold at partitions 0..99 in a (100, 9, 91)
            # layout, then one SBUF->SBUF DMA into layout partitions 100..119
            mx = sm.tile([P, 9], f32, tag="mx")
            nc.vector.tensor_reduce(
                out=mx[:100, :],
                in_=cc[:100, :].rearrange("p (c k) -> p c k", k=C),
                axis=mybir.AxisListType.X, op=Alu.max,
            )
            bitsp = sm.tile([P, 9], f32, tag="bitsp")
            nc.vector.tensor_scalar(
                out=bitsp[:100, :], in0=mx[:100, :], scalar1=THR, scalar2=None,
                op0=Alu.is_ge,
            )
            nc.sync.dma_start(
                out=bits[100:120, :], in_=bitsp[:100, :]
            )

            # within-partition inclusive prefix
            sc = sm.tile([P, S], f32, tag="sc")
            nc.vector.tensor_tensor_scan(
                out=sc[:], data0=bits[:], data1=bits[:], initial=0.0,
                op0=Alu.add, op1=Alu.bypass,
            )

            # cross-partition inclusive prefix of per-partition totals (exact):
            # transpose totals to one partition, scan, transpose back.
            segt_ps = ps.tile([1, P], f32, tag="segt")
            nc.tensor.transpose(out=segt_ps[:], in_=sc[:, S - 1 : S], identity=ident[:])
            segt = sm.tile([1, P], f32, tag="segs")
            nc.vector.tensor_copy(segt[:], segt_ps[:])
            pinc_t = sm.tile([1, P], f32, tag="pinct")
            nc.vector.tensor_tensor_scan(
                out=pinc_t[:], data0=segt[:], data1=segt[:], initial=0.0,
                op0=Alu.add, op1=Alu.bypass,
            )
            pinc_ps = ps.tile([P, 1], f32, tag="pinc")
            nc.tensor.transpose(out=pinc_ps[:], in_=pinc_t[:], identity=ident[:1, :1])
            pinc = sm.tile([P, 1], f32, tag="pincs")
            nc.vector.tensor_copy(pinc[:], pinc_ps[:])

            # exclusive offset (+ -1 for 0-based dest): off = pinc - seg_tot - 1
            off = sm.tile([P, 1], f32, tag="off")
            nc.vector.scalar_tensor_tensor(
                out=off[:], in0=pinc[:], scalar=-1.0, in1=sc[:, S - 1 : S],
                op0=Alu.add, op1=Alu.subtract,
            )

            # dest = sc + off where bit set, else BIG
            dpos = sm.tile([P, S], f32, tag="dpos")
            nc.vector.scalar_tensor_tensor(
                out=dpos[:], in0=sc[:], scalar=off[:, :1], in1=sc[:],
                op0=Alu.add, op1=Alu.bypass,
            )
            bitsi = sm.tile([P, S], i32, tag="bitsi")
            nc.vector.tensor_copy(bitsi[:], bits[:])
            idxf = sm.tile([P, S], f32, tag="idxf")
            nc.vector.memset(idxf[:], BIG)
            nc.vector.copy_predicated(out=idxf[:], mask=bitsi[:], data=dpos[:])
            idxi = sm.tile([P, S], i32, tag="idxi")
            nc.vector.tensor_copy(idxi[:], idxf[:])

            # nb_items = min(total, M), broadcast to all partitions (exact matmul)
            totm = sm.tile([P, 1], f32, tag="totm")
            nc.vector.tensor_scalar(
                out=totm[:], in0=pinc[:], scalar1=float(M), scalar2=None, op0=Alu.min
            )
            totb_ps = ps.tile([P, 1], f32, tag="totb")
            nc.tensor.matmul(totb_ps[:], lhsT=e127[:], rhs=totm[:], start=True, stop=True)
            totb = sm.tile([P, 1], f32, tag="totbs")
            nc.vector.tensor_copy(totb[:], totb_ps[:])

            # mask output: j < nb_items (only partitions 0..99 hold j < 4500)
            mkf = sm.tile([P, S], f32, tag="mkf")
            nc.vector.scalar_tensor_tensor(
                out=mkf[:], in0=jtf[:], scalar=totb[:, :1], in1=jtf[:],
                op0=Alu.is_lt, op1=Alu.bypass,
            )
            mk8 = sm.tile([P, S], u8, tag="mk8")
            nc.vector.tensor_copy(mk8[:], mkf[:])
            nc.sync.dma_start(
                out=outm[be].rearrange("(p c) -> p c", c=S), in_=mk8[:100, :]
            )

            nbi = sm.tile([1, 1], i32, tag="nbi")
            nc.vector.tensor_copy(nbi[:], totb[:1, :1])
            nc.sync.dma_start(out=outn[be], in_=nbi[:])
            return wv, idxi

        # main scatter: surviving rows -> compacted destinations.
        # HW SWDGE supports one index per partition, so one indirect DMA per
        # column (128 rows each); dropped rows and capacity overflow are
        # skipped via the OOB bounds check. Tail rows stay pre-zeroed.
        # Process elements in pairs with interleaved column scatters so each
        # chain's DMA-completion wait hides behind the other chain's call.
        G = 3
        for grp in range(0, nbe, G):
            fronts = [(be, *element_front(be)) for be in range(grp, min(grp + G, nbe))]
            for c in range(S):
                for be, wv, idxi in fronts:
                    nc.gpsimd.indirect_dma_start(
                        out=outw_t[be][:],
                        out_offset=bass.IndirectOffsetOnAxis(
                            ap=idxi[:, c : c + 1], axis=0
                        ),
                        in_=wv[:, c, :],
                        in_offset=None,
                        bounds_check=breg,
                        oob_is_err=False,
                    )

    nc.compile()
    return nc


_NC_CACHE = {}


def _get_nc():
    if "nc" not in _NC_CACHE:
        _NC_CACHE["nc"] = build(NBE)
    return _NC_CACHE["nc"]


def _pack_core(mem_emb, fn, tid, msk, p_emb, p_cls, p_id, cur):
    """Pack one core's shard (NBE elements) into kernel input arrays."""
    nbe = mem_emb.shape[0]
    srcw = np.zeros((nbe, R, RW), np.float32)
    srcw[:, :M, :D] = mem_emb
    srcw[:, :M, 256] = np.ascontiguousarray(fn).view(np.float32)
    srcw[:, :M, 257] = np.ascontiguousarray(tid).view(np.float32)
    srcw[:, :M, 258] = fn.astype(np.float32)
    srcw[:, :M, 259] = msk.astype(np.float32)
    srcw[:, M : M + Q, :D] = p_emb
    curb = np.ascontiguousarray(np.broadcast_to(cur[:, None], (nbe, Q)).astype(np.int32))
    srcw[:, M : M + Q, 256] = curb.view(np.float32)
    srcw[:, M : M + Q, 257] = np.ascontiguousarray(p_id).view(np.float32)
    clsx = np.ascontiguousarray(p_cls.reshape(nbe, 100, 9 * C))
    thrx = np.ascontiguousarray(
        np.broadcast_to((cur.astype(np.float32) - 5.0)[:, None, None], (nbe, P, 1))
    )
    return {"srcw": srcw.reshape(nbe * R, RW), "cls": clsx, "thr": thrx}


def _unpack_core(res, nbe=NBE):
    ow = np.stack([res[f"outw{b}"] for b in range(nbe)], axis=0)
    emb = np.ascontiguousarray(ow[:, :, :D])
    fn = np.ascontiguousarray(ow[:, :, 256]).view(np.int32)
    tid = np.ascontiguousarray(ow[:, :, 257]).view(np.int32)
    mask = res["outm"].astype(bool)
    nb = res["outn"].reshape(nbe).astype(np.int32)
    fn = np.where(mask, fn, -1)  # tail rows keep the buffer's zero fill; fn tail is -1
    return emb, fn, tid, mask, nb


def run(inputs, trace=False, trace_kwargs=None):
    """Shard across 8 cores, run the bass kernel, gather. Returns
    ((emb, fn, tid, mask, nb_items), BassKernelResults)."""
    nc = _get_nc()
    in_maps = []
    for c in range(N_CORES):
        sl = slice(c * NBE, (c + 1) * NBE)
        in_maps.append(
            _pack_core(
                inputs["mem_embeddings"][sl],
                inputs["mem_frame_numbers"][sl],
                inputs["mem_tracking_ids"][sl],
                inputs["mem_mask"][sl],
                inputs["pred_embedds"][sl],
                inputs["pred_classes"][sl],
                inputs["pred_ids"][sl],
                inputs["current_frames"][sl],
            )
        )
    br = run_bass_kernel_spmd(
        nc, in_maps, core_ids=list(range(N_CORES)), trace=trace,
        **({"trace_kwargs": trace_kwargs} if trace_kwargs else {}),
    )
    parts = [_unpack_core(r) for r in br.results]
    emb = np.concatenate([p[0] for p in parts], axis=0)
    fn = np.concatenate([p[1] for p in parts], axis=0)
    tid = np.concatenate([p[2] for p in parts], axis=0)
    mask = np.concatenate([p[3] for p in parts], axis=0)
    nb = np.concatenate([p[4] for p in parts], axis=0)
    return (emb, fn, tid, mask, nb), br


def kernel(**inputs):
    outs, _ = run(inputs, trace=False)
    return outs
